# revision 19
# baseline (speedup 1.0000x reference)
"""Trainium2 Bass kernel for nn_FPLayer (retrieval_knn):
cdist -> top-3 -> inverse-distance feature interpolation -> pointwise MLP with sync-BN.

Sharding: data-parallel over batch B=8 across 8 NeuronCores (1 batch each).
BatchNorm batch stats are all-reduced across cores (sync-BN).

End-to-end latency through the axon-tunneled PJRT path is dominated by
host<->device transfer bytes, so ALL inputs are packed into a single bf16
tensor per core (f32/fp8 sections live in it via bitcast views) and the
output is bf16:
  - feat1/feat2 are shipped as int8 with per-channel quantization scales
    folded into W0's input columns on the host (so the device computes with
    raw int values, which are exact in bf16). The remaining bf16 rounding is
    suppressed by hi/lo-splitting the MLP weights and layer-1 activations.
  - the [8192,128] result is written as bf16 and upcast on host.

Per-core device pipeline:
  - cdist via PE matmul with an augmented contraction: v = 2*<x1,x2> - |x2|^2
    computed with 27 bf16 hi/mid/lo split rows (error ~ fp32 ulp) at full bf16
    rate; the exactness keeps top-3 selection faithful to the fp32 reference.
  - top-8 via DVE max8 + max_index directly on PSUM (fp32, exact).
  - weights w_k = (1/(d_k+1e-8)) / sum via small batched vector ops.
  - feature gather via gpsimd indirect DMA (bf16 row gather from DRAM).
  - interp = sum_k w_k * gathered_k via scalar_tensor_tensor.
  - MLP computed in transposed domain (channels on partitions); bf16 matmuls
    with fp32 PSUM accumulation, weights as hi+lo bf16 pairs.
  - BN stats per channel = per-partition sums (ACT accum_out for S1, ACT
    Square for S2), AllReduce'd across the 8 cores; BN+ReLU fused ACT pass.
"""

import numpy as np

import jax

# The per-call wall time through run_bass_kernel_spmd is dominated by host
# work and transfers; without a persistent compilation cache, every call
# re-runs BIR verification + DVE table generation (~0.5s) because the jit
# closure is rebuilt per call. With the cache, repeat calls load the
# NEFF-wrapped executable directly.
jax.config.update("jax_compilation_cache_dir", "/tmp/jax_comp_cache")
jax.config.update("jax_persistent_cache_min_compile_time_secs", 0.0)
jax.config.update("jax_persistent_cache_min_entry_size_bytes", 0)

import concourse.bass as bass
import concourse.mybir as mybir
import concourse.tile as tile
from concourse import bacc
from concourse.bass_utils import run_bass_kernel_spmd

B, N1, N2, C1, C2 = 8, 8192, 2048, 128, 256
MLP0, MLP1 = 256, 128
KNN = 3
BN_EPS = 1e-5
NT = N1 // 128          # 64 row tiles per core
NG = 16                 # groups of 4 tiles (512 rows)
GT = NT // NG           # tiles per group = 4
KAUG = 27               # augmented contraction rows
F32 = mybir.dt.float32
BF16 = mybir.dt.bfloat16
I8 = mybir.dt.int8
U32 = mybir.dt.uint32

# packed bf16 tensor layout (bf16-element offsets; f32/int8 sections bitcast)
SZ_F2 = N2 * C2 // 2            # feat2 int8 (first: indirect-DMA src offset 0)
SZ_F1 = 128 * N1 // 2           # feat1T as int8 bytes (2 int8 per bf16 slot)
SZ_XYZ1 = 3 * N1 * 2            # xyz1^T f32 (coord-major); splits built on device
SZ_XYZ2 = 3 * N2 * 2
SZ_W0 = 128 * 6 * MLP0          # W0^T hi+lo chunks
SZ_W1 = 128 * 4 * MLP1          # W1^T hi+lo chunks
SZ_SQ1 = 128 * NT * 2           # f32 as 2 bf16 slots
SZ_BNP0 = 128 * 4 * 2
SZ_BNP1 = 128 * 2 * 2
OFF_F2 = 0
OFF_F1 = OFF_F2 + SZ_F2
OFF_XYZ1 = OFF_F1 + SZ_F1
OFF_XYZ2 = OFF_XYZ1 + SZ_XYZ1
OFF_W0 = OFF_XYZ2 + SZ_XYZ2
OFF_W1 = OFF_W0 + SZ_W0
OFF_SQ1 = OFF_W1 + SZ_W1
OFF_BNP0 = OFF_SQ1 + SZ_SQ1
OFF_BNP1 = OFF_BNP0 + SZ_BNP0
TOT16 = OFF_BNP1 + SZ_BNP1

_prog_cache = {}
_last_in_maps = None


def _split2(x):
    """Split fp32 array into hi/lo bf16 parts a+b ~= x (error ~2^-16 rel)."""
    import ml_dtypes
    bf = ml_dtypes.bfloat16
    a = x.astype(bf).astype(np.float32)
    b = (x - a).astype(bf)
    return a.astype(bf), b


def _split3(x):
    """Split fp32 array into 3 bf16 parts a+b+c ~= x (error ~2^-24 rel)."""
    import ml_dtypes
    bf = ml_dtypes.bfloat16
    a = x.astype(bf).astype(np.float32)
    b = (x - a).astype(bf).astype(np.float32)
    c = (x - a - b).astype(bf).astype(np.float32)
    return a, b, c


def _host_prep(xyz1b, xyz2b, feat1b, feat2b, W0, W1, bnp0, bnp1):
    """Build the packed per-core input. Returns dict with one array.

    feat1/feat2 are quantized to int8 with per-channel scales; the scales are
    folded into W0's input columns, so the device sees raw int values (exact
    in bf16) and the matmul output is identical to using s*q floats.
    """
    import ml_dtypes
    bf = ml_dtypes.bfloat16
    s1 = np.maximum(np.abs(feat1b).max(0), 1e-12).astype(np.float32) / 127.0   # [128]
    q1 = np.clip(np.rint(feat1b / s1), -127, 127).astype(np.int8)              # [8192,128]
    s2 = np.maximum(np.abs(feat2b).max(0), 1e-12).astype(np.float32) / 127.0   # [256]
    q2 = np.clip(np.rint(feat2b / s2), -127, 127).astype(np.int8)              # [2048,256]
    W0s = W0 * np.concatenate([s1, s2])[None, :]                               # [256,384]
    w0h, w0l = _split2(W0s.T)
    w0t = np.ascontiguousarray(
        np.concatenate([w0h.reshape(3, 128, MLP0), w0l.reshape(3, 128, MLP0)], 0).transpose(1, 0, 2))
    w1h, w1l = _split2(W1.T)
    w1h = w1h.reshape(2, 128, MLP1)
    w1l = w1l.reshape(2, 128, MLP1)
    w1t = np.ascontiguousarray(np.stack([w1h[0], w1l[0], w1h[1], w1l[1]], 0).transpose(1, 0, 2))

    sq1 = (xyz1b.astype(np.float32) ** 2).sum(-1).astype(np.float32)
    sq1t = np.ascontiguousarray(sq1.reshape(NT, 128).T)  # [128, NT]
    feat1T8 = np.ascontiguousarray(q1.T)          # [128, N1] int8
    xyz1c = np.ascontiguousarray(xyz1b.T.astype(np.float32))  # [3, N1]
    xyz2c = np.ascontiguousarray(xyz2b.T.astype(np.float32))  # [3, N2]

    pk16 = np.empty((TOT16,), bf)
    pk16[OFF_F2:OFF_F2 + SZ_F2] = np.ascontiguousarray(q2).ravel().view(bf)
    pk16[OFF_F1:OFF_F1 + SZ_F1] = feat1T8.ravel().view(bf)
    pk16[OFF_XYZ1:OFF_XYZ1 + SZ_XYZ1] = xyz1c.ravel().view(bf)
    pk16[OFF_XYZ2:OFF_XYZ2 + SZ_XYZ2] = xyz2c.ravel().view(bf)
    pk16[OFF_W0:OFF_W0 + SZ_W0] = w0t.ravel()
    pk16[OFF_W1:OFF_W1 + SZ_W1] = w1t.ravel()
    pk16[OFF_SQ1:OFF_SQ1 + SZ_SQ1] = sq1t.ravel().view(bf)
    pk16[OFF_BNP0:OFF_BNP0 + SZ_BNP0] = bnp0.ravel().view(bf)
    pk16[OFF_BNP1:OFF_BNP1 + SZ_BNP1] = bnp1.ravel().view(bf)
    return {"pk16": pk16}


def _build_program(n_cores):
    nc = bacc.Bacc("TRN2", target_bir_lowering=False, debug=False)

    pk16_d = nc.dram_tensor("pk16", [TOT16], BF16, kind="ExternalInput")
    out_d = nc.dram_tensor("out", [N1, MLP1], BF16, kind="ExternalOutput")

    feat2_v = pk16_d[OFF_F2:OFF_F2 + SZ_F2].bitcast(I8).rearrange("(a b) -> a b", b=C2)  # [2048, 256] int8
    feat1_v = pk16_d[OFF_F1:OFF_F1 + SZ_F1].bitcast(I8).rearrange("(a b) -> a b", b=N1)  # [128, 8192] int8
    xyz1c_v = pk16_d[OFF_XYZ1:OFF_XYZ1 + SZ_XYZ1].bitcast(F32).rearrange("(a b) -> a b", b=N1)  # [3, 8192]
    xyz2c_v = pk16_d[OFF_XYZ2:OFF_XYZ2 + SZ_XYZ2].bitcast(F32).rearrange("(a b) -> a b", b=N2)  # [3, 2048]
    w0t_v = pk16_d[OFF_W0:OFF_W0 + SZ_W0].rearrange("(a b c) -> a b c", b=6, c=MLP0)
    w1t_v = pk16_d[OFF_W1:OFF_W1 + SZ_W1].rearrange("(a b c) -> a b c", b=4, c=MLP1)
    sq1t_v = pk16_d[OFF_SQ1:OFF_SQ1 + SZ_SQ1].bitcast(F32).rearrange("(a b) -> a b", b=NT)
    bnp0_v = pk16_d[OFF_BNP0:OFF_BNP0 + SZ_BNP0].bitcast(F32).rearrange("(a b) -> a b", b=4)
    bnp1_v = pk16_d[OFF_BNP1:OFF_BNP1 + SZ_BNP1].bitcast(F32).rearrange("(a b) -> a b", b=2)

    NTOT = float(B * N1)  # total rows across cores for BN stats

    with tile.TileContext(nc) as tc:
        with (
            tc.tile_pool(name="const", bufs=1) as cpool,
            tc.tile_pool(name="karr", bufs=1) as kpool,
            tc.tile_pool(name="vps", bufs=1, space="PSUM") as vps_pool,
            tc.tile_pool(name="tps", bufs=2, space="PSUM") as tps_pool,
            tc.tile_pool(name="mps", bufs=1, space="PSUM") as mps_pool,
            tc.tile_pool(name="gbuf", bufs=2) as gpool,
            tc.tile_pool(name="xbuf", bufs=1) as xpool,
            tc.tile_pool(name="sbuf", bufs=2) as spool,
            tc.tile_pool(name="dram", bufs=1, space="DRAM") as dram,
        ):
            # ---- constants / persistent ----
            x1s = cpool.tile([KAUG, N1], BF16)
            x2s = cpool.tile([KAUG, N2], BF16)
            sq1t = cpool.tile([128, NT], F32)
            w0t = cpool.tile([128, 6, MLP0], BF16)
            w1t = cpool.tile([128, 4, MLP1], BF16)
            bnp0 = cpool.tile([128, 4], F32)
            bnp1 = cpool.tile([128, 2], F32)
            ident = cpool.tile([128, 128], F32)
            nc.sync.dma_start(sq1t[:], sq1t_v)
            nc.sync.dma_start(w0t[:], w0t_v)
            nc.sync.dma_start(w1t[:], w1t_v)
            nc.sync.dma_start(bnp0[:], bnp0_v)
            nc.sync.dma_start(bnp1[:], bnp1_v)
            from concourse.masks import make_identity
            make_identity(nc, ident[:])

            # ---- build the 27 augmented cdist rows from raw xyz on device ----
            # pair layout per coord c (row = 8c+i):
            #   i: 0:(a1,d2) 1:(a1,e2) 2:(b1,d2) 3:(a1,f2) 4:(b1,e2) 5:(c1,d2) 6:(b1,f2) 7:(c1,e2)
            # rows 24-26: (ones, 3-split of -sq2)
            # Compute engines must start at partition 0, so each part is
            # computed in partition-0 staging tiles and DMA-placed into rows.
            CH = N1 // 2
            xyzrow = kpool.tile([1, CH], F32)
            resr = kpool.tile([1, CH], F32)
            ast = kpool.tile([1, CH], BF16)
            bst = kpool.tile([1, CH], BF16)
            cst = kpool.tile([1, CH], BF16)
            sq2r = kpool.tile([1, N2], F32)
            onest = kpool.tile([1, CH], BF16)
            nc.vector.memset(onest[:], 1.0)
            for c in range(3):
                r = 8 * c
                for h in range(2):
                    sl = slice(h * CH, (h + 1) * CH)
                    nc.sync.dma_start(xyzrow[:], xyz1c_v[c:c + 1, sl])
                    nc.vector.tensor_scalar(out=ast[:], in0=xyzrow[:], scalar1=2.0,
                                            scalar2=None, op0=mybir.AluOpType.mult)      # a1 = bf16(2x)
                    nc.vector.scalar_tensor_tensor(out=resr[:], in0=xyzrow[:], scalar=2.0, in1=ast[:],
                                                   op0=mybir.AluOpType.mult, op1=mybir.AluOpType.subtract)
                    nc.vector.tensor_copy(bst[:], resr[:])                               # b1
                    nc.vector.tensor_tensor(out=resr[:], in0=resr[:], in1=bst[:], op=mybir.AluOpType.subtract)
                    nc.vector.tensor_copy(cst[:], resr[:])                               # c1
                    for dst in (r, r + 1, r + 3):
                        nc.sync.dma_start(x1s[dst:dst + 1, sl], ast[:])
                    for dst in (r + 2, r + 4, r + 6):
                        nc.sync.dma_start(x1s[dst:dst + 1, sl], bst[:])
                    for dst in (r + 5, r + 7):
                        nc.sync.dma_start(x1s[dst:dst + 1, sl], cst[:])
                    if c == 0:
                        for dst in (24, 25, 26):
                            nc.sync.dma_start(x1s[dst:dst + 1, sl], onest[:])
            for c in range(3):
                r = 8 * c
                x2row = xyzrow[:, 0:N2]
                res2 = resr[:, 0:N2]
                tmp2 = resr[:, N2:2 * N2]
                a2 = ast[:, 0:N2]
                b2 = bst[:, 0:N2]
                c2 = cst[:, 0:N2]
                nc.sync.dma_start(x2row, xyz2c_v[c:c + 1, :])
                nc.vector.tensor_copy(a2, x2row)                                         # d2
                nc.vector.tensor_tensor(out=res2, in0=x2row, in1=a2, op=mybir.AluOpType.subtract)
                nc.vector.tensor_copy(b2, res2)                                          # e2
                nc.vector.tensor_tensor(out=res2, in0=res2, in1=b2, op=mybir.AluOpType.subtract)
                nc.vector.tensor_copy(c2, res2)                                          # f2
                if c == 0:
                    nc.vector.tensor_tensor(out=sq2r[:], in0=x2row, in1=x2row, op=mybir.AluOpType.mult)
                else:
                    nc.vector.tensor_tensor(out=tmp2, in0=x2row, in1=x2row, op=mybir.AluOpType.mult)
                    nc.vector.tensor_tensor(out=sq2r[:], in0=sq2r[:], in1=tmp2, op=mybir.AluOpType.add)
                for dst in (r, r + 2, r + 5):
                    nc.sync.dma_start(x2s[dst:dst + 1, :], a2)
                for dst in (r + 1, r + 4, r + 7):
                    nc.sync.dma_start(x2s[dst:dst + 1, :], b2)
                for dst in (r + 3, r + 6):
                    nc.sync.dma_start(x2s[dst:dst + 1, :], c2)
            a2 = ast[:, 0:N2]
            b2 = bst[:, 0:N2]
            c2 = cst[:, 0:N2]
            nc.vector.tensor_scalar(out=a2, in0=sq2r[:], scalar1=-1.0,
                                    scalar2=None, op0=mybir.AluOpType.mult)              # sa = bf16(-sq2)
            nc.vector.tensor_scalar(out=sq2r[:], in0=sq2r[:], scalar1=-1.0,
                                    scalar2=None, op0=mybir.AluOpType.mult)
            nc.vector.tensor_tensor(out=sq2r[:], in0=sq2r[:], in1=a2, op=mybir.AluOpType.subtract)
            nc.vector.tensor_copy(b2, sq2r[:])                                           # sb
            nc.vector.tensor_tensor(out=sq2r[:], in0=sq2r[:], in1=b2, op=mybir.AluOpType.subtract)
            nc.vector.tensor_copy(c2, sq2r[:])                                           # sc
            nc.sync.dma_start(x2s[24:25, :], a2)
            nc.sync.dma_start(x2s[25:26, :], b2)
            nc.sync.dma_start(x2s[26:27, :], c2)

            mv_all = cpool.tile([128, NT, 8], F32)
            mi_all = cpool.tile([128, NT, 8], U32)

            # ================= Phase 1: KNN =================
            for t in range(NT):
                v_ps = vps_pool.tile([128, N2], F32, tag="v")
                for j in range(4):
                    nc.tensor.matmul(
                        v_ps[:, j * 512:(j + 1) * 512],
                        x1s[:, t * 128:(t + 1) * 128],
                        x2s[:, j * 512:(j + 1) * 512],
                        start=True, stop=True,
                    )
                nc.vector.max(out=mv_all[:, t, :], in_=v_ps[:])
                nc.vector.max_index(out=mi_all[:, t, :], in_max=mv_all[:, t, :], in_values=v_ps[:])

            # ---- batched weight computation ----
            # d2 = sq1 - v   (v = 2cross - sq2)
            mv3 = mv_all[:, :, 0:KNN]                      # [128, NT, 3]
            d2 = kpool.tile([128, NT, KNN], F32)
            nc.vector.tensor_tensor(out=d2[:], in0=sq1t[:].to_broadcast([128, NT, KNN]),
                                    in1=mv3, op=mybir.AluOpType.subtract)
            nc.vector.tensor_scalar_max(d2[:], d2[:], 1e-12)
            dist = kpool.tile([128, NT, KNN], F32)
            nc.scalar.activation(out=dist[:], in_=d2[:], func=mybir.ActivationFunctionType.Sqrt)
            nc.vector.tensor_scalar_add(dist[:], dist[:], 1e-8)
            rr = kpool.tile([128, NT, KNN], F32)
            nc.vector.reciprocal(out=rr[:], in_=dist[:])
            rs = kpool.tile([128, NT, 1], F32)
            nc.vector.tensor_reduce(out=rs[:], in_=rr[:], axis=mybir.AxisListType.X, op=mybir.AluOpType.add)
            rsr = kpool.tile([128, NT, 1], F32)
            nc.vector.reciprocal(out=rsr[:], in_=rs[:])
            w_all = kpool.tile([128, NT, KNN], F32)
            nc.vector.tensor_tensor(out=w_all[:], in0=rr[:], in1=rsr[:].to_broadcast([128, NT, KNN]),
                                    op=mybir.AluOpType.mult)

            # contiguous per-k index arrays for indirect DMA offsets
            mi_k = kpool.tile([128, KNN, NT], U32)
            for k in range(KNN):
                nc.vector.tensor_copy(mi_k[:, k, :], mi_all[:, :, k])

            # ================= Phase 2: gather + interp + transposed MLP =================
            x0T = []
            for c in range(2):
                x0Tc = xpool.tile([128, N1], BF16, tag=f"x0T{c}", name=f"x0T{c}")
                x0T.append(x0Tc)
            x1T = xpool.tile([128, N1], BF16, tag="x1T")
            s1p0 = kpool.tile([128, 2, NG], F32)   # per-(chunk, group) sums of x0
            s2p0 = kpool.tile([128, 2, NG], F32)
            s1p1 = kpool.tile([128, NG], F32)
            s2p1 = kpool.tile([128, NG], F32)
            nc.vector.memset(s1p0[:], 0.0)
            nc.vector.memset(s2p0[:], 0.0)
            nc.vector.memset(s1p1[:], 0.0)
            nc.vector.memset(s2p1[:], 0.0)

            for g in range(NG):
                # gathers for this group's 4 tiles (one indirect DMA per (tile, k))
                gk = []
                for k in range(KNN):
                    gt = gpool.tile([128, GT, C2], I8, tag=f"g{k}", name=f"g{k}")
                    for j in range(GT):
                        t = g * GT + j
                        nc.gpsimd.indirect_dma_start(
                            out=gt[:, j, :],
                            out_offset=None,
                            in_=feat2_v,
                            in_offset=bass.IndirectOffsetOnAxis(ap=mi_k[:, k, t:t + 1], axis=0),
                        )
                    gk.append(gt)
                # feat1 int8 -> bf16 expansion (values <=127: exact in bf16);
                # per-channel dequant scales are folded into W0's columns.
                inT = gpool.tile([128, 3, 512], BF16, tag="inT")
                f1i8 = gpool.tile([128, 512], I8, tag="f1i8")
                nc.sync.dma_start(f1i8[:], feat1_v[:, g * 512:(g + 1) * 512])
                nc.scalar.activation(out=inT[:, 0, :], in_=f1i8[:],
                                     func=mybir.ActivationFunctionType.Copy)
                # weighted interp per tile, then transpose to channel-major
                for j in range(GT):
                    t = g * GT + j
                    itp = gpool.tile([128, C2], F32, tag="itp")
                    nc.vector.tensor_scalar(out=itp[:], in0=gk[0][:, j, :], scalar1=w_all[:, t, 0:1],
                                            scalar2=None, op0=mybir.AluOpType.mult)
                    nc.vector.scalar_tensor_tensor(out=itp[:], in0=gk[1][:, j, :], scalar=w_all[:, t, 1:2],
                                                   in1=itp[:], op0=mybir.AluOpType.mult, op1=mybir.AluOpType.add)
                    nc.vector.scalar_tensor_tensor(out=itp[:], in0=gk[2][:, j, :], scalar=w_all[:, t, 2:3],
                                                   in1=itp[:], op0=mybir.AluOpType.mult, op1=mybir.AluOpType.add)
                    for c in range(2):
                        tp = tps_pool.tile([128, 128], F32, tag="tp")
                        nc.tensor.transpose(out=tp[:], in_=itp[:, c * 128:(c + 1) * 128], identity=ident[:])
                        nc.scalar.activation(out=inT[:, 1 + c, j * 128:(j + 1) * 128], in_=tp[:],
                                             func=mybir.ActivationFunctionType.Copy)

                # layer 0 matmuls: x0T chunk [128 out_ch, 512 rows]; W0 as hi+lo
                for c in range(2):
                    x0ps = mps_pool.tile([128, 512], F32, tag="x0ps")
                    for ki in range(6):
                        nc.tensor.matmul(
                            x0ps[:],
                            w0t[:, ki, c * 128:(c + 1) * 128],
                            inT[:, ki % 3, :],
                            start=(ki == 0), stop=(ki == 5),
                        )
                    # S2 partial via ACT Square with accumulate; S1 fused into the copy
                    junk = spool.tile([128, 512], BF16, tag="junk")
                    nc.scalar.activation(out=junk[:], in_=x0ps[:], func=mybir.ActivationFunctionType.Square,
                                         accum_out=s2p0[:, c, g:g + 1])
                    nc.scalar.activation(out=x0T[c][:, g * 512:(g + 1) * 512], in_=x0ps[:],
                                         func=mybir.ActivationFunctionType.Copy,
                                         accum_out=s1p0[:, c, g:g + 1])

            # ---- BN0: reduce partials, AllReduce, compute affine ----
            st0 = kpool.tile([128, 4], F32)
            nc.vector.tensor_reduce(out=st0[:, 0:1], in_=s1p0[:, 0, :], axis=mybir.AxisListType.X, op=mybir.AluOpType.add)
            nc.vector.tensor_reduce(out=st0[:, 1:2], in_=s2p0[:, 0, :], axis=mybir.AxisListType.X, op=mybir.AluOpType.add)
            nc.vector.tensor_reduce(out=st0[:, 2:3], in_=s1p0[:, 1, :], axis=mybir.AxisListType.X, op=mybir.AluOpType.add)
            nc.vector.tensor_reduce(out=st0[:, 3:4], in_=s2p0[:, 1, :], axis=mybir.AxisListType.X, op=mybir.AluOpType.add)
            st0_in = dram.tile([128, 4], F32)
            st0_out = dram.tile([128, 4], F32)
            nc.gpsimd.dma_start(st0_in[:], st0[:])
            nc.gpsimd.collective_compute(
                "AllReduce", mybir.AluOpType.add,
                replica_groups=[list(range(n_cores))],
                ins=[st0_in.opt()], outs=[st0_out.opt()],
            )
            st0g = kpool.tile([128, 4], F32)
            nc.sync.dma_start(st0g[:], st0_out[:])
            # mean/var -> a = g*rsqrt(var+eps), bb = be - mean*a   (per chunk)
            ab0 = kpool.tile([128, 4], F32)   # a_c0, b_c0, a_c1, b_c1
            mean0 = kpool.tile([128, 2], F32)
            var0 = kpool.tile([128, 2], F32)
            sd0 = kpool.tile([128, 2], F32)
            m20 = kpool.tile([128, 2], F32)
            for c in range(2):
                nc.vector.tensor_scalar_mul(mean0[:, c:c + 1], st0g[:, 2 * c:2 * c + 1], 1.0 / NTOT)
                nc.vector.tensor_scalar_mul(var0[:, c:c + 1], st0g[:, 2 * c + 1:2 * c + 2], 1.0 / NTOT)
            nc.vector.tensor_tensor(out=m20[:], in0=mean0[:], in1=mean0[:], op=mybir.AluOpType.mult)
            nc.vector.tensor_tensor(out=var0[:], in0=var0[:], in1=m20[:], op=mybir.AluOpType.subtract)
            nc.vector.tensor_scalar_add(var0[:], var0[:], BN_EPS)
            nc.scalar.activation(out=sd0[:], in_=var0[:], func=mybir.ActivationFunctionType.Sqrt)
            nc.vector.reciprocal(out=sd0[:], in_=sd0[:])
            for c in range(2):
                nc.vector.tensor_tensor(out=ab0[:, 2 * c:2 * c + 1], in0=bnp0[:, 2 * c:2 * c + 1],
                                        in1=sd0[:, c:c + 1], op=mybir.AluOpType.mult)
                nc.vector.scalar_tensor_tensor(out=ab0[:, 2 * c + 1:2 * c + 2], in0=mean0[:, c:c + 1],
                                               scalar=-1.0, in1=ab0[:, 2 * c:2 * c + 1],
                                               op0=mybir.AluOpType.mult, op1=mybir.AluOpType.mult)
                nc.vector.tensor_tensor(out=ab0[:, 2 * c + 1:2 * c + 2], in0=ab0[:, 2 * c + 1:2 * c + 2],
                                        in1=bnp0[:, 2 * c + 1:2 * c + 2], op=mybir.AluOpType.add)

            # ---- layer 1 (+ BN1 stats); x0n and W1 as hi+lo ----
            for g in range(NG):
                x0h, x0l = [], []
                for c in range(2):
                    x0nf = spool.tile([128, 512], F32, tag=f"x0nf{c}", name=f"x0nf{c}")
                    nc.scalar.activation(out=x0nf[:], in_=x0T[c][:, g * 512:(g + 1) * 512],
                                         func=mybir.ActivationFunctionType.Relu,
                                         scale=ab0[:, 2 * c:2 * c + 1], bias=ab0[:, 2 * c + 1:2 * c + 2])
                    xh = spool.tile([128, 512], BF16, tag=f"x0h{c}", name=f"x0h{c}")
                    xl = spool.tile([128, 512], BF16, tag=f"x0l{c}", name=f"x0l{c}")
                    nc.vector.tensor_copy(xh[:], x0nf[:])
                    nc.vector.tensor_tensor(out=xl[:], in0=x0nf[:], in1=xh[:], op=mybir.AluOpType.subtract)
                    x0h.append(xh)
                    x0l.append(xl)
                x1ps = mps_pool.tile([128, 512], F32, tag="x1ps")
                # terms: wh*xh + wh*xl + wl*xh  (wl*xl ~ 2^-32, dropped)
                mms = []
                for c in range(2):
                    mms += [(w1t[:, 2 * c, :], x0h[c]), (w1t[:, 2 * c, :], x0l[c]),
                            (w1t[:, 2 * c + 1, :], x0h[c])]
                for i, (wv, xv) in enumerate(mms):
                    nc.tensor.matmul(x1ps[:], wv, xv[:], start=(i == 0), stop=(i == len(mms) - 1))
                junk = spool.tile([128, 512], BF16, tag="junk")
                nc.scalar.activation(out=junk[:], in_=x1ps[:], func=mybir.ActivationFunctionType.Square,
                                     accum_out=s2p1[:, g:g + 1])
                nc.scalar.activation(out=x1T[:, g * 512:(g + 1) * 512], in_=x1ps[:],
                                     func=mybir.ActivationFunctionType.Copy,
                                     accum_out=s1p1[:, g:g + 1])

            # ---- BN1 ----
            st1 = kpool.tile([128, 2], F32)
            nc.vector.tensor_reduce(out=st1[:, 0:1], in_=s1p1[:], axis=mybir.AxisListType.X, op=mybir.AluOpType.add)
            nc.vector.tensor_reduce(out=st1[:, 1:2], in_=s2p1[:], axis=mybir.AxisListType.X, op=mybir.AluOpType.add)
            st1_in = dram.tile([128, 2], F32)
            st1_out = dram.tile([128, 2], F32)
            nc.gpsimd.dma_start(st1_in[:], st1[:])
            nc.gpsimd.collective_compute(
                "AllReduce", mybir.AluOpType.add,
                replica_groups=[list(range(n_cores))],
                ins=[st1_in.opt()], outs=[st1_out.opt()],
            )
            st1g = kpool.tile([128, 2], F32)
            nc.sync.dma_start(st1g[:], st1_out[:])
            ab1 = kpool.tile([128, 2], F32)
            mean1 = kpool.tile([128, 1], F32)
            var1 = kpool.tile([128, 1], F32)
            nc.vector.tensor_scalar_mul(mean1[:], st1g[:, 0:1], 1.0 / NTOT)
            nc.vector.tensor_scalar_mul(var1[:], st1g[:, 1:2], 1.0 / NTOT)
            m21 = kpool.tile([128, 1], F32)
            nc.vector.tensor_tensor(out=m21[:], in0=mean1[:], in1=mean1[:], op=mybir.AluOpType.mult)
            nc.vector.tensor_tensor(out=var1[:], in0=var1[:], in1=m21[:], op=mybir.AluOpType.subtract)
            nc.vector.tensor_scalar_add(var1[:], var1[:], BN_EPS)
            nc.scalar.activation(out=var1[:], in_=var1[:], func=mybir.ActivationFunctionType.Sqrt)
            nc.vector.reciprocal(out=var1[:], in_=var1[:])
            nc.vector.tensor_tensor(out=ab1[:, 0:1], in0=bnp1[:, 0:1], in1=var1[:], op=mybir.AluOpType.mult)
            nc.vector.scalar_tensor_tensor(out=ab1[:, 1:2], in0=mean1[:], scalar=-1.0, in1=ab1[:, 0:1],
                                           op0=mybir.AluOpType.mult, op1=mybir.AluOpType.mult)
            nc.vector.tensor_tensor(out=ab1[:, 1:2], in0=ab1[:, 1:2], in1=bnp1[:, 1:2], op=mybir.AluOpType.add)

            # ---- BN1 apply + final transpose + output (bf16) ----
            for g in range(NG):
                x2t = spool.tile([128, 512], F32, tag="x2t")
                nc.scalar.activation(out=x2t[:], in_=x1T[:, g * 512:(g + 1) * 512],
                                     func=mybir.ActivationFunctionType.Relu,
                                     scale=ab1[:, 0:1], bias=ab1[:, 1:2])
                for j in range(GT):
                    t = g * GT + j
                    tp = tps_pool.tile([128, 128], F32, tag="tp")
                    nc.tensor.transpose(out=tp[:], in_=x2t[:, j * 128:(j + 1) * 128], identity=ident[:])
                    onat = spool.tile([128, 128], BF16, tag="onat")
                    nc.vector.tensor_copy(onat[:], tp[:])
                    nc.sync.dma_start(out_d[t * 128:(t + 1) * 128, :], onat[:])

    nc.compile()
    return nc


def _get_program(n_cores):
    if n_cores not in _prog_cache:
        _prog_cache[n_cores] = _build_program(n_cores)
    return _prog_cache[n_cores]


def _prep_shared(gamma0, beta0, gamma1, beta1):
    bnp0 = np.stack([np.asarray(gamma0[:128]), np.asarray(beta0[:128]),
                     np.asarray(gamma0[128:]), np.asarray(beta0[128:])], 1).astype(np.float32)
    bnp1 = np.stack([np.asarray(gamma1), np.asarray(beta1)], 1).astype(np.float32)
    return bnp0, bnp1


def kernel(xyz1, xyz2, feat1, feat2, W0, b0, gamma0, beta0, W1, b1, gamma1, beta1):
    # note: b0/b1 cancel exactly inside train-mode BatchNorm -> ignored.
    xyz1 = np.asarray(xyz1, np.float32)
    xyz2 = np.asarray(xyz2, np.float32)
    feat1 = np.asarray(feat1, np.float32)
    feat2 = np.asarray(feat2, np.float32)
    W0 = np.asarray(W0, np.float32)
    W1 = np.asarray(W1, np.float32)
    bnp0, bnp1 = _prep_shared(gamma0, beta0, gamma1, beta1)

    n_cores = B
    nc = _get_program(n_cores)
    in_maps = []
    for b in range(B):
        in_maps.append(_host_prep(xyz1[b], xyz2[b], feat1[b], feat2[b], W0, W1, bnp0, bnp1))

    global _last_in_maps
    _last_in_maps = in_maps
    res = run_bass_kernel_spmd(nc, in_maps, list(range(n_cores)))
    out = np.stack([res.results[b]["out"] for b in range(B)], 0).astype(np.float32)
    return out


# revision 34
# speedup vs baseline: 1.1023x; 1.1023x over previous
"""Trainium2 Bass kernel for nn_FPLayer (retrieval_knn):
cdist -> top-3 -> inverse-distance feature interpolation -> pointwise MLP with sync-BN.

Sharding: data-parallel over batch B=8 across 8 NeuronCores (1 batch each).
BatchNorm batch stats are all-reduced across cores (sync-BN).

End-to-end latency through the axon-tunneled PJRT path is dominated by
host<->device transfer bytes, so ALL inputs are packed into a single bf16
tensor per core (f32/fp8 sections live in it via bitcast views) and the
output is bf16:
  - feat1/feat2 are shipped as int8 with per-channel quantization scales
    folded into W0's input columns on the host (so the device computes with
    raw int values, which are exact in fp16). The MLP runs in fp16 (11-bit
    mantissa) so weight/activation rounding is negligible without splits.
  - xyz is shipped raw (f32, coord-major); the augmented cdist rows are
    built on device as fp16 hi/lo pairs.
  - the [8192,128] result is written as bf16 and upcast on host.

Per-core device pipeline:
  - cdist via PE matmul with an augmented contraction: v = 2*<x1,x2> - |x2|^2
    computed with 27 bf16 hi/mid/lo split rows (error ~ fp32 ulp) at full bf16
    rate; the exactness keeps top-3 selection faithful to the fp32 reference.
  - top-8 via DVE max8 + max_index directly on PSUM (fp32, exact).
  - weights w_k = (1/(d_k+1e-8)) / sum via small batched vector ops.
  - feature gather via gpsimd indirect DMA (bf16 row gather from DRAM).
  - interp = sum_k w_k * gathered_k via scalar_tensor_tensor.
  - MLP computed in transposed domain (channels on partitions); bf16 matmuls
    with fp32 PSUM accumulation, weights as hi+lo bf16 pairs.
  - BN stats per channel = per-partition sums (ACT accum_out for S1, ACT
    Square for S2), AllReduce'd across the 8 cores; BN+ReLU fused ACT pass.
"""

import numpy as np

import jax

# The per-call wall time through run_bass_kernel_spmd is dominated by host
# work and transfers; without a persistent compilation cache, every call
# re-runs BIR verification + DVE table generation (~0.5s) because the jit
# closure is rebuilt per call. With the cache, repeat calls load the
# NEFF-wrapped executable directly.
jax.config.update("jax_compilation_cache_dir", "/tmp/jax_comp_cache")
jax.config.update("jax_persistent_cache_min_compile_time_secs", 0.0)
jax.config.update("jax_persistent_cache_min_entry_size_bytes", 0)

import concourse.bass as bass
import concourse.mybir as mybir
import concourse.tile as tile
from concourse import bacc
from concourse.bass_utils import run_bass_kernel_spmd

B, N1, N2, C1, C2 = 8, 8192, 2048, 128, 256
MLP0, MLP1 = 256, 128
KNN = 3
BN_EPS = 1e-5
NT = N1 // 128          # 64 row tiles per core
NG = 16                 # groups of 4 tiles (512 rows)
GT = NT // NG           # tiles per group = 4
KAUG = 21               # augmented contraction rows (fp16 3-split pairs + sq2)
F32 = mybir.dt.float32
BF16 = mybir.dt.bfloat16
F16 = mybir.dt.float16
I8 = mybir.dt.int8
U32 = mybir.dt.uint32

# packed bf16 tensor layout (bf16-element offsets; f32/int8 sections bitcast)
SZ_F2 = N2 * C2 // 2            # feat2 int8 (first: indirect-DMA src offset 0)
SZ_F1 = 128 * N1 // 2           # feat1T as int8 bytes (2 int8 per bf16 slot)
SZ_XYZ1 = 3 * N1 * 2            # xyz1^T f32 (coord-major); splits built on device
SZ_XYZ2 = 3 * N2 * 2
SZ_W0 = 128 * 3 * MLP0          # W0^T fp16 chunks
SZ_W1 = 128 * 2 * MLP1          # W1^T fp16 chunks
SZ_SQ1 = 128 * NT * 2           # f32 as 2 bf16 slots
SZ_BNP0 = 128 * 4 * 2
SZ_BNP1 = 128 * 2 * 2
OFF_F2 = 0
OFF_F1 = OFF_F2 + SZ_F2
OFF_XYZ1 = OFF_F1 + SZ_F1
OFF_XYZ2 = OFF_XYZ1 + SZ_XYZ1
OFF_W0 = OFF_XYZ2 + SZ_XYZ2
OFF_W1 = OFF_W0 + SZ_W0
OFF_SQ1 = OFF_W1 + SZ_W1
OFF_BNP0 = OFF_SQ1 + SZ_SQ1
OFF_BNP1 = OFF_BNP0 + SZ_BNP0
TOT16 = OFF_BNP1 + SZ_BNP1

_prog_cache = {}
_last_in_maps = None


def _host_prep(xyz1b, xyz2b, feat1b, feat2b, W0, W1, bnp0, bnp1):
    """Build the packed per-core input. Returns dict with one array.

    feat1/feat2 are quantized to int8 with per-channel scales; the scales are
    folded into W0's input columns, so the device sees raw int values (exact
    in bf16) and the matmul output is identical to using s*q floats.
    """
    import ml_dtypes
    bf = ml_dtypes.bfloat16
    s1 = np.maximum(np.abs(feat1b).max(0), 1e-12).astype(np.float32) / 127.0   # [128]
    q1 = np.clip(np.rint(feat1b / s1), -127, 127).astype(np.int8)              # [8192,128]
    s2 = np.maximum(np.abs(feat2b).max(0), 1e-12).astype(np.float32) / 127.0   # [256]
    q2 = np.clip(np.rint(feat2b / s2), -127, 127).astype(np.int8)              # [2048,256]
    W0s = W0 * np.concatenate([s1, s2])[None, :]                               # [256,384]
    w0t = np.ascontiguousarray(
        W0s.T.astype(np.float16).reshape(3, 128, MLP0).transpose(1, 0, 2))     # [128,3,256]
    w1t = np.ascontiguousarray(
        W1.T.astype(np.float16).reshape(2, 128, MLP1).transpose(1, 0, 2))      # [128,2,128]

    sq1 = (xyz1b.astype(np.float32) ** 2).sum(-1).astype(np.float32)
    sq1t = np.ascontiguousarray(sq1.reshape(NT, 128).T)  # [128, NT]
    feat1T8 = np.ascontiguousarray(q1.T)          # [128, N1] int8
    xyz1c = np.ascontiguousarray(xyz1b.T.astype(np.float32))  # [3, N1]
    xyz2c = np.ascontiguousarray(xyz2b.T.astype(np.float32))  # [3, N2]

    pk16 = np.empty((TOT16,), bf)
    pk16[OFF_F2:OFF_F2 + SZ_F2] = np.ascontiguousarray(q2).ravel().view(bf)
    pk16[OFF_F1:OFF_F1 + SZ_F1] = feat1T8.ravel().view(bf)
    pk16[OFF_XYZ1:OFF_XYZ1 + SZ_XYZ1] = xyz1c.ravel().view(bf)
    pk16[OFF_XYZ2:OFF_XYZ2 + SZ_XYZ2] = xyz2c.ravel().view(bf)
    pk16[OFF_W0:OFF_W0 + SZ_W0] = w0t.ravel().view(bf)
    pk16[OFF_W1:OFF_W1 + SZ_W1] = w1t.ravel().view(bf)
    pk16[OFF_SQ1:OFF_SQ1 + SZ_SQ1] = sq1t.ravel().view(bf)
    pk16[OFF_BNP0:OFF_BNP0 + SZ_BNP0] = bnp0.ravel().view(bf)
    pk16[OFF_BNP1:OFF_BNP1 + SZ_BNP1] = bnp1.ravel().view(bf)
    return {"pk16": pk16}


def _build_program(n_cores):
    nc = bacc.Bacc("TRN2", target_bir_lowering=False, debug=False)

    pk16_d = nc.dram_tensor("pk16", [TOT16], BF16, kind="ExternalInput")
    out_d = nc.dram_tensor("out", [N1, MLP1], BF16, kind="ExternalOutput")

    feat2_v = pk16_d[OFF_F2:OFF_F2 + SZ_F2].bitcast(I8).rearrange("(a b) -> a b", b=C2)  # [2048, 256] int8
    feat1_v = pk16_d[OFF_F1:OFF_F1 + SZ_F1].bitcast(I8).rearrange("(a b) -> a b", b=N1)  # [128, 8192] int8
    xyz1c_v = pk16_d[OFF_XYZ1:OFF_XYZ1 + SZ_XYZ1].bitcast(F32).rearrange("(a b) -> a b", b=N1)  # [3, 8192]
    xyz2c_v = pk16_d[OFF_XYZ2:OFF_XYZ2 + SZ_XYZ2].bitcast(F32).rearrange("(a b) -> a b", b=N2)  # [3, 2048]
    w0t_v = pk16_d[OFF_W0:OFF_W0 + SZ_W0].bitcast(F16).rearrange("(a b c) -> a b c", b=3, c=MLP0)
    w1t_v = pk16_d[OFF_W1:OFF_W1 + SZ_W1].bitcast(F16).rearrange("(a b c) -> a b c", b=2, c=MLP1)
    sq1t_v = pk16_d[OFF_SQ1:OFF_SQ1 + SZ_SQ1].bitcast(F32).rearrange("(a b) -> a b", b=NT)
    bnp0_v = pk16_d[OFF_BNP0:OFF_BNP0 + SZ_BNP0].bitcast(F32).rearrange("(a b) -> a b", b=4)
    bnp1_v = pk16_d[OFF_BNP1:OFF_BNP1 + SZ_BNP1].bitcast(F32).rearrange("(a b) -> a b", b=2)

    NTOT = float(B * N1)  # total rows across cores for BN stats

    with tile.TileContext(nc) as tc:
        with (
            tc.tile_pool(name="const", bufs=1) as cpool,
            tc.tile_pool(name="karr", bufs=1) as kpool,
            tc.tile_pool(name="vps", bufs=1, space="PSUM") as vps_pool,
            tc.tile_pool(name="tps", bufs=2, space="PSUM") as tps_pool,
            tc.tile_pool(name="mps", bufs=1, space="PSUM") as mps_pool,
            tc.tile_pool(name="gbuf", bufs=2) as gpool,
            tc.tile_pool(name="xbuf", bufs=1) as xpool,
            tc.tile_pool(name="sbuf", bufs=2) as spool,
            tc.tile_pool(name="dram", bufs=1, space="DRAM") as dram,
        ):
            # ---- constants / persistent ----
            x1s = cpool.tile([KAUG, N1], F16)
            x2s = cpool.tile([KAUG, N2], F16)
            sq1t = cpool.tile([128, NT], F32)
            w0t = cpool.tile([128, 3, MLP0], F16)
            w1t = cpool.tile([128, 2, MLP1], F16)
            bnp0 = cpool.tile([128, 4], F32)
            bnp1 = cpool.tile([128, 2], F32)
            ident = cpool.tile([128, 128], F32)
            nc.sync.dma_start(sq1t[:], sq1t_v)
            nc.sync.dma_start(w0t[:], w0t_v)
            nc.sync.dma_start(w1t[:], w1t_v)
            nc.sync.dma_start(bnp0[:], bnp0_v)
            nc.sync.dma_start(bnp1[:], bnp1_v)
            from concourse.masks import make_identity
            make_identity(nc, ident[:])

            # ---- build the 21 augmented cdist rows from raw xyz on device ----
            # fp16 3-split per side: a+b+c ~= x (residual ~2^-33). Six pair
            # rows per coord recover 2<x1,x2> to fp32 ulp; pair layout per
            # coord (row = 6c+i):
            #   i: 0:(a1,d2) 1:(a1,e2) 2:(a1,f2) 3:(b1,d2) 4:(b1,e2) 5:(c1,d2)
            # rows 18-20: (ones, 3-split of -sq2)
            # Compute engines must start at partition 0, so each part is
            # computed in partition-0 staging tiles and DMA-placed into rows.
            CH = N1 // 2
            xyzrow = kpool.tile([1, CH], F32)
            resr = kpool.tile([1, CH], F32)
            ast = kpool.tile([1, CH], F16)
            bst = kpool.tile([1, CH], F16)
            cst = kpool.tile([1, CH], F16)
            sq2r = kpool.tile([1, N2], F32)
            onest = kpool.tile([1, CH], F16)
            nc.vector.memset(onest[:], 1.0)
            for c in range(3):
                r = 6 * c
                for h in range(2):
                    sl = slice(h * CH, (h + 1) * CH)
                    nc.sync.dma_start(xyzrow[:], xyz1c_v[c:c + 1, sl])
                    nc.vector.tensor_scalar(out=ast[:], in0=xyzrow[:], scalar1=2.0,
                                            scalar2=None, op0=mybir.AluOpType.mult)      # a1 = fp16(2x)
                    nc.vector.scalar_tensor_tensor(out=resr[:], in0=xyzrow[:], scalar=2.0, in1=ast[:],
                                                   op0=mybir.AluOpType.mult, op1=mybir.AluOpType.subtract)
                    nc.vector.tensor_copy(bst[:], resr[:])                               # b1
                    nc.vector.tensor_tensor(out=resr[:], in0=resr[:], in1=bst[:], op=mybir.AluOpType.subtract)
                    nc.vector.tensor_copy(cst[:], resr[:])                               # c1
                    for dst in (r, r + 1, r + 2):
                        nc.sync.dma_start(x1s[dst:dst + 1, sl], ast[:])
                    for dst in (r + 3, r + 4):
                        nc.sync.dma_start(x1s[dst:dst + 1, sl], bst[:])
                    nc.sync.dma_start(x1s[r + 5:r + 6, sl], cst[:])
                    if c == 0:
                        for dst in (18, 19, 20):
                            nc.sync.dma_start(x1s[dst:dst + 1, sl], onest[:])
            for c in range(3):
                r = 6 * c
                x2row = xyzrow[:, 0:N2]
                res2 = resr[:, 0:N2]
                tmp2 = resr[:, N2:2 * N2]
                a2 = ast[:, 0:N2]
                b2 = bst[:, 0:N2]
                c2 = cst[:, 0:N2]
                nc.sync.dma_start(x2row, xyz2c_v[c:c + 1, :])
                nc.vector.tensor_copy(a2, x2row)                                         # d2 = fp16(x)
                nc.vector.tensor_tensor(out=res2, in0=x2row, in1=a2, op=mybir.AluOpType.subtract)
                nc.vector.tensor_copy(b2, res2)                                          # e2
                nc.vector.tensor_tensor(out=res2, in0=res2, in1=b2, op=mybir.AluOpType.subtract)
                nc.vector.tensor_copy(c2, res2)                                          # f2
                if c == 0:
                    nc.vector.tensor_tensor(out=sq2r[:], in0=x2row, in1=x2row, op=mybir.AluOpType.mult)
                else:
                    nc.vector.tensor_tensor(out=tmp2, in0=x2row, in1=x2row, op=mybir.AluOpType.mult)
                    nc.vector.tensor_tensor(out=sq2r[:], in0=sq2r[:], in1=tmp2, op=mybir.AluOpType.add)
                for dst in (r, r + 3, r + 5):
                    nc.sync.dma_start(x2s[dst:dst + 1, :], a2)
                for dst in (r + 1, r + 4):
                    nc.sync.dma_start(x2s[dst:dst + 1, :], b2)
                nc.sync.dma_start(x2s[r + 2:r + 3, :], c2)
            a2 = ast[:, 0:N2]
            b2 = bst[:, 0:N2]
            c2 = cst[:, 0:N2]
            nc.vector.tensor_scalar(out=a2, in0=sq2r[:], scalar1=-1.0,
                                    scalar2=None, op0=mybir.AluOpType.mult)              # sa = fp16(-sq2)
            nc.vector.tensor_scalar(out=sq2r[:], in0=sq2r[:], scalar1=-1.0,
                                    scalar2=None, op0=mybir.AluOpType.mult)
            nc.vector.tensor_tensor(out=sq2r[:], in0=sq2r[:], in1=a2, op=mybir.AluOpType.subtract)
            nc.vector.tensor_copy(b2, sq2r[:])                                           # sb
            nc.vector.tensor_tensor(out=sq2r[:], in0=sq2r[:], in1=b2, op=mybir.AluOpType.subtract)
            nc.vector.tensor_copy(c2, sq2r[:])                                           # sc
            nc.sync.dma_start(x2s[18:19, :], a2)
            nc.sync.dma_start(x2s[19:20, :], b2)
            nc.sync.dma_start(x2s[20:21, :], c2)

            mv_all = cpool.tile([128, NT, 8], F32)
            mi_all = cpool.tile([128, NT, 8], U32)

            # ================= Phase 1: KNN =================
            for t in range(NT):
                v_ps = vps_pool.tile([128, N2], F32, tag="v")
                for j in range(4):
                    nc.tensor.matmul(
                        v_ps[:, j * 512:(j + 1) * 512],
                        x1s[:, t * 128:(t + 1) * 128],
                        x2s[:, j * 512:(j + 1) * 512],
                        start=True, stop=True,
                    )
                nc.vector.max(out=mv_all[:, t, :], in_=v_ps[:])
                nc.vector.max_index(out=mi_all[:, t, :], in_max=mv_all[:, t, :], in_values=v_ps[:])

            # ---- batched weight computation ----
            # d2 = sq1 - v   (v = 2cross - sq2)
            mv3 = mv_all[:, :, 0:KNN]                      # [128, NT, 3]
            d2 = kpool.tile([128, NT, KNN], F32)
            nc.vector.tensor_tensor(out=d2[:], in0=sq1t[:].to_broadcast([128, NT, KNN]),
                                    in1=mv3, op=mybir.AluOpType.subtract)
            nc.vector.tensor_scalar_max(d2[:], d2[:], 1e-12)
            dist = kpool.tile([128, NT, KNN], F32)
            nc.scalar.activation(out=dist[:], in_=d2[:], func=mybir.ActivationFunctionType.Sqrt)
            nc.vector.tensor_scalar_add(dist[:], dist[:], 1e-8)
            rr = kpool.tile([128, NT, KNN], F32)
            nc.vector.reciprocal(out=rr[:], in_=dist[:])
            rs = kpool.tile([128, NT, 1], F32)
            nc.vector.tensor_reduce(out=rs[:], in_=rr[:], axis=mybir.AxisListType.X, op=mybir.AluOpType.add)
            rsr = kpool.tile([128, NT, 1], F32)
            nc.vector.reciprocal(out=rsr[:], in_=rs[:])
            w_all = kpool.tile([128, NT, KNN], F32)
            nc.vector.tensor_tensor(out=w_all[:], in0=rr[:], in1=rsr[:].to_broadcast([128, NT, KNN]),
                                    op=mybir.AluOpType.mult)

            # contiguous per-k index arrays for indirect DMA offsets
            mi_k = kpool.tile([128, KNN, NT], U32)
            for k in range(KNN):
                nc.vector.tensor_copy(mi_k[:, k, :], mi_all[:, :, k])

            # ================= Phase 2: gather + interp + transposed MLP =================
            x0T = []
            for c in range(2):
                x0Tc = xpool.tile([128, N1], F16, tag=f"x0T{c}", name=f"x0T{c}")
                x0T.append(x0Tc)
            x1T = xpool.tile([128, N1], F16, tag="x1T")
            s1p0 = kpool.tile([128, 2, NG], F32)   # per-(chunk, group) sums of x0
            s2p0 = kpool.tile([128, 2, NG], F32)
            s1p1 = kpool.tile([128, NG], F32)
            s2p1 = kpool.tile([128, NG], F32)
            nc.vector.memset(s1p0[:], 0.0)
            nc.vector.memset(s2p0[:], 0.0)
            nc.vector.memset(s1p1[:], 0.0)
            nc.vector.memset(s2p1[:], 0.0)

            for g in range(NG):
                # gathers for this group's 4 tiles (one indirect DMA per (tile, k))
                gk = []
                for k in range(KNN):
                    gt = gpool.tile([128, GT, C2], I8, tag=f"g{k}", name=f"g{k}")
                    for j in range(GT):
                        t = g * GT + j
                        nc.gpsimd.indirect_dma_start(
                            out=gt[:, j, :],
                            out_offset=None,
                            in_=feat2_v,
                            in_offset=bass.IndirectOffsetOnAxis(ap=mi_k[:, k, t:t + 1], axis=0),
                        )
                    gk.append(gt)
                # feat1 int8 -> fp16 expansion (values <=127: exact);
                # per-channel dequant scales are folded into W0's columns.
                inT = gpool.tile([128, 3, 512], F16, tag="inT")
                f1i8 = gpool.tile([128, 512], I8, tag="f1i8")
                nc.sync.dma_start(f1i8[:], feat1_v[:, g * 512:(g + 1) * 512])
                nc.scalar.activation(out=inT[:, 0, :], in_=f1i8[:],
                                     func=mybir.ActivationFunctionType.Copy)
                # weighted interp per tile, then transpose to channel-major
                for j in range(GT):
                    t = g * GT + j
                    itp = gpool.tile([128, C2], F32, tag="itp")
                    nc.vector.tensor_scalar(out=itp[:], in0=gk[0][:, j, :], scalar1=w_all[:, t, 0:1],
                                            scalar2=None, op0=mybir.AluOpType.mult)
                    nc.vector.scalar_tensor_tensor(out=itp[:], in0=gk[1][:, j, :], scalar=w_all[:, t, 1:2],
                                                   in1=itp[:], op0=mybir.AluOpType.mult, op1=mybir.AluOpType.add)
                    nc.vector.scalar_tensor_tensor(out=itp[:], in0=gk[2][:, j, :], scalar=w_all[:, t, 2:3],
                                                   in1=itp[:], op0=mybir.AluOpType.mult, op1=mybir.AluOpType.add)
                    for c in range(2):
                        tp = tps_pool.tile([128, 128], F32, tag="tp")
                        nc.tensor.transpose(out=tp[:], in_=itp[:, c * 128:(c + 1) * 128], identity=ident[:])
                        nc.scalar.activation(out=inT[:, 1 + c, j * 128:(j + 1) * 128], in_=tp[:],
                                             func=mybir.ActivationFunctionType.Copy)

                # layer 0 matmuls: x0T chunk [128 out_ch, 512 rows]
                for c in range(2):
                    x0ps = mps_pool.tile([128, 512], F32, tag="x0ps")
                    for ki in range(3):
                        nc.tensor.matmul(
                            x0ps[:],
                            w0t[:, ki, c * 128:(c + 1) * 128],
                            inT[:, ki, :],
                            start=(ki == 0), stop=(ki == 2),
                        )
                    # S2 partial via ACT Square with accumulate; S1 fused into the copy
                    junk = spool.tile([128, 512], BF16, tag="junk")
                    nc.scalar.activation(out=junk[:], in_=x0ps[:], func=mybir.ActivationFunctionType.Square,
                                         accum_out=s2p0[:, c, g:g + 1])
                    nc.scalar.activation(out=x0T[c][:, g * 512:(g + 1) * 512], in_=x0ps[:],
                                         func=mybir.ActivationFunctionType.Copy,
                                         accum_out=s1p0[:, c, g:g + 1])

            # ---- BN0: reduce partials, AllReduce, compute affine ----
            st0 = kpool.tile([128, 4], F32)
            nc.vector.tensor_reduce(out=st0[:, 0:1], in_=s1p0[:, 0, :], axis=mybir.AxisListType.X, op=mybir.AluOpType.add)
            nc.vector.tensor_reduce(out=st0[:, 1:2], in_=s2p0[:, 0, :], axis=mybir.AxisListType.X, op=mybir.AluOpType.add)
            nc.vector.tensor_reduce(out=st0[:, 2:3], in_=s1p0[:, 1, :], axis=mybir.AxisListType.X, op=mybir.AluOpType.add)
            nc.vector.tensor_reduce(out=st0[:, 3:4], in_=s2p0[:, 1, :], axis=mybir.AxisListType.X, op=mybir.AluOpType.add)
            st0_in = dram.tile([128, 4], F32)
            st0_out = dram.tile([128, 4], F32)
            nc.gpsimd.dma_start(st0_in[:], st0[:])
            nc.gpsimd.collective_compute(
                "AllReduce", mybir.AluOpType.add,
                replica_groups=[list(range(n_cores))],
                ins=[st0_in.opt()], outs=[st0_out.opt()],
            )
            st0g = kpool.tile([128, 4], F32)
            nc.sync.dma_start(st0g[:], st0_out[:])
            # mean/var -> a = g*rsqrt(var+eps), bb = be - mean*a   (per chunk)
            ab0 = kpool.tile([128, 4], F32)   # a_c0, b_c0, a_c1, b_c1
            mean0 = kpool.tile([128, 2], F32)
            var0 = kpool.tile([128, 2], F32)
            sd0 = kpool.tile([128, 2], F32)
            m20 = kpool.tile([128, 2], F32)
            for c in range(2):
                nc.vector.tensor_scalar_mul(mean0[:, c:c + 1], st0g[:, 2 * c:2 * c + 1], 1.0 / NTOT)
                nc.vector.tensor_scalar_mul(var0[:, c:c + 1], st0g[:, 2 * c + 1:2 * c + 2], 1.0 / NTOT)
            nc.vector.tensor_tensor(out=m20[:], in0=mean0[:], in1=mean0[:], op=mybir.AluOpType.mult)
            nc.vector.tensor_tensor(out=var0[:], in0=var0[:], in1=m20[:], op=mybir.AluOpType.subtract)
            nc.vector.tensor_scalar_add(var0[:], var0[:], BN_EPS)
            nc.scalar.activation(out=sd0[:], in_=var0[:], func=mybir.ActivationFunctionType.Sqrt)
            nc.vector.reciprocal(out=sd0[:], in_=sd0[:])
            for c in range(2):
                nc.vector.tensor_tensor(out=ab0[:, 2 * c:2 * c + 1], in0=bnp0[:, 2 * c:2 * c + 1],
                                        in1=sd0[:, c:c + 1], op=mybir.AluOpType.mult)
                nc.vector.scalar_tensor_tensor(out=ab0[:, 2 * c + 1:2 * c + 2], in0=mean0[:, c:c + 1],
                                               scalar=-1.0, in1=ab0[:, 2 * c:2 * c + 1],
                                               op0=mybir.AluOpType.mult, op1=mybir.AluOpType.mult)
                nc.vector.tensor_tensor(out=ab0[:, 2 * c + 1:2 * c + 2], in0=ab0[:, 2 * c + 1:2 * c + 2],
                                        in1=bnp0[:, 2 * c + 1:2 * c + 2], op=mybir.AluOpType.add)

            # ---- layer 1 (+ BN1 stats) ----
            for g in range(NG):
                x0n = []
                for c in range(2):
                    x0nc = spool.tile([128, 512], F16, tag=f"x0n{c}", name=f"x0n{c}")
                    nc.scalar.activation(out=x0nc[:], in_=x0T[c][:, g * 512:(g + 1) * 512],
                                         func=mybir.ActivationFunctionType.Relu,
                                         scale=ab0[:, 2 * c:2 * c + 1], bias=ab0[:, 2 * c + 1:2 * c + 2])
                    x0n.append(x0nc)
                x1ps = mps_pool.tile([128, 512], F32, tag="x1ps")
                for c in range(2):
                    nc.tensor.matmul(x1ps[:], w1t[:, c, :], x0n[c][:], start=(c == 0), stop=(c == 1))
                junk = spool.tile([128, 512], BF16, tag="junk")
                nc.scalar.activation(out=junk[:], in_=x1ps[:], func=mybir.ActivationFunctionType.Square,
                                     accum_out=s2p1[:, g:g + 1])
                nc.scalar.activation(out=x1T[:, g * 512:(g + 1) * 512], in_=x1ps[:],
                                     func=mybir.ActivationFunctionType.Copy,
                                     accum_out=s1p1[:, g:g + 1])

            # ---- BN1 ----
            st1 = kpool.tile([128, 2], F32)
            nc.vector.tensor_reduce(out=st1[:, 0:1], in_=s1p1[:], axis=mybir.AxisListType.X, op=mybir.AluOpType.add)
            nc.vector.tensor_reduce(out=st1[:, 1:2], in_=s2p1[:], axis=mybir.AxisListType.X, op=mybir.AluOpType.add)
            st1_in = dram.tile([128, 2], F32)
            st1_out = dram.tile([128, 2], F32)
            nc.gpsimd.dma_start(st1_in[:], st1[:])
            nc.gpsimd.collective_compute(
                "AllReduce", mybir.AluOpType.add,
                replica_groups=[list(range(n_cores))],
                ins=[st1_in.opt()], outs=[st1_out.opt()],
            )
            st1g = kpool.tile([128, 2], F32)
            nc.sync.dma_start(st1g[:], st1_out[:])
            ab1 = kpool.tile([128, 2], F32)
            mean1 = kpool.tile([128, 1], F32)
            var1 = kpool.tile([128, 1], F32)
            nc.vector.tensor_scalar_mul(mean1[:], st1g[:, 0:1], 1.0 / NTOT)
            nc.vector.tensor_scalar_mul(var1[:], st1g[:, 1:2], 1.0 / NTOT)
            m21 = kpool.tile([128, 1], F32)
            nc.vector.tensor_tensor(out=m21[:], in0=mean1[:], in1=mean1[:], op=mybir.AluOpType.mult)
            nc.vector.tensor_tensor(out=var1[:], in0=var1[:], in1=m21[:], op=mybir.AluOpType.subtract)
            nc.vector.tensor_scalar_add(var1[:], var1[:], BN_EPS)
            nc.scalar.activation(out=var1[:], in_=var1[:], func=mybir.ActivationFunctionType.Sqrt)
            nc.vector.reciprocal(out=var1[:], in_=var1[:])
            nc.vector.tensor_tensor(out=ab1[:, 0:1], in0=bnp1[:, 0:1], in1=var1[:], op=mybir.AluOpType.mult)
            nc.vector.scalar_tensor_tensor(out=ab1[:, 1:2], in0=mean1[:], scalar=-1.0, in1=ab1[:, 0:1],
                                           op0=mybir.AluOpType.mult, op1=mybir.AluOpType.mult)
            nc.vector.tensor_tensor(out=ab1[:, 1:2], in0=ab1[:, 1:2], in1=bnp1[:, 1:2], op=mybir.AluOpType.add)

            # ---- BN1 apply + final transpose + output (bf16) ----
            for g in range(NG):
                x2t = spool.tile([128, 512], F32, tag="x2t")
                nc.scalar.activation(out=x2t[:], in_=x1T[:, g * 512:(g + 1) * 512],
                                     func=mybir.ActivationFunctionType.Relu,
                                     scale=ab1[:, 0:1], bias=ab1[:, 1:2])
                for j in range(GT):
                    t = g * GT + j
                    tp = tps_pool.tile([128, 128], F32, tag="tp")
                    nc.tensor.transpose(out=tp[:], in_=x2t[:, j * 128:(j + 1) * 128], identity=ident[:])
                    onat = spool.tile([128, 128], BF16, tag="onat")
                    nc.vector.tensor_copy(onat[:], tp[:])
                    nc.sync.dma_start(out_d[t * 128:(t + 1) * 128, :], onat[:])

    nc.compile()
    return nc


def _get_program(n_cores):
    if n_cores not in _prog_cache:
        _prog_cache[n_cores] = _build_program(n_cores)
    return _prog_cache[n_cores]


def _prep_shared(gamma0, beta0, gamma1, beta1):
    bnp0 = np.stack([np.asarray(gamma0[:128]), np.asarray(beta0[:128]),
                     np.asarray(gamma0[128:]), np.asarray(beta0[128:])], 1).astype(np.float32)
    bnp1 = np.stack([np.asarray(gamma1), np.asarray(beta1)], 1).astype(np.float32)
    return bnp0, bnp1


def kernel(xyz1, xyz2, feat1, feat2, W0, b0, gamma0, beta0, W1, b1, gamma1, beta1):
    # note: b0/b1 cancel exactly inside train-mode BatchNorm -> ignored.
    xyz1 = np.asarray(xyz1, np.float32)
    xyz2 = np.asarray(xyz2, np.float32)
    feat1 = np.asarray(feat1, np.float32)
    feat2 = np.asarray(feat2, np.float32)
    W0 = np.asarray(W0, np.float32)
    W1 = np.asarray(W1, np.float32)
    bnp0, bnp1 = _prep_shared(gamma0, beta0, gamma1, beta1)

    n_cores = B
    nc = _get_program(n_cores)
    in_maps = []
    for b in range(B):
        in_maps.append(_host_prep(xyz1[b], xyz2[b], feat1[b], feat2[b], W0, W1, bnp0, bnp1))

    global _last_in_maps
    _last_in_maps = in_maps
    res = run_bass_kernel_spmd(nc, in_maps, list(range(n_cores)))
    out = np.stack([res.results[b]["out"] for b in range(B)], 0).astype(np.float32)
    return out


# revision 46
# speedup vs baseline: 1.5168x; 1.3761x over previous
"""Trainium2 Bass kernel for nn_FPLayer (retrieval_knn):
cdist -> top-3 -> inverse-distance feature interpolation -> pointwise MLP with sync-BN.

Sharding: data-parallel over batch B=8 across 8 NeuronCores (1 batch each).
BatchNorm batch stats are all-reduced across cores (sync-BN).

End-to-end latency through the axon-tunneled PJRT path is dominated by
host<->device transfer bytes, so ALL inputs are packed into a single bf16
tensor per core (f32/fp8 sections live in it via bitcast views) and the
output is bf16:
  - feat1/feat2 are shipped as int8 with per-channel quantization scales
    folded into W0's input columns on the host (so the device computes with
    raw int values, which are exact in fp16). The MLP runs in fp16 (11-bit
    mantissa) so weight/activation rounding is negligible without splits.
  - xyz is shipped raw (f32, coord-major); the augmented cdist rows are
    built on device as fp16 hi/lo pairs.
  - the [8192,128] result is quantized to uint8 on device (the post-ReLU
    output has a known per-channel range; round-to-nearest is done exactly
    via the fp32 +2^23 magic-number trick) and dequantized on host.

Per-core device pipeline:
  - cdist via PE matmul with an augmented contraction: v = 2*<x1,x2> - |x2|^2
    computed with 27 bf16 hi/mid/lo split rows (error ~ fp32 ulp) at full bf16
    rate; the exactness keeps top-3 selection faithful to the fp32 reference.
  - top-8 via DVE max8 + max_index directly on PSUM (fp32, exact).
  - weights w_k = (1/(d_k+1e-8)) / sum via small batched vector ops.
  - feature gather via gpsimd indirect DMA (bf16 row gather from DRAM).
  - interp = sum_k w_k * gathered_k via scalar_tensor_tensor.
  - MLP computed in transposed domain (channels on partitions); bf16 matmuls
    with fp32 PSUM accumulation, weights as hi+lo bf16 pairs.
  - BN stats per channel = per-partition sums (ACT accum_out for S1, ACT
    Square for S2), AllReduce'd across the 8 cores; BN+ReLU fused ACT pass.
"""

import numpy as np

import jax

# The per-call wall time through run_bass_kernel_spmd is dominated by host
# work and transfers; without a persistent compilation cache, every call
# re-runs BIR verification + DVE table generation (~0.5s) because the jit
# closure is rebuilt per call. With the cache, repeat calls load the
# NEFF-wrapped executable directly.
jax.config.update("jax_compilation_cache_dir", "/tmp/jax_comp_cache")
jax.config.update("jax_persistent_cache_min_compile_time_secs", 0.0)
jax.config.update("jax_persistent_cache_min_entry_size_bytes", 0)

import concourse.bass as bass
import concourse.mybir as mybir
import concourse.tile as tile
from concourse import bacc
from concourse.bass_utils import run_bass_kernel_spmd

B, N1, N2, C1, C2 = 8, 8192, 2048, 128, 256
MLP0, MLP1 = 256, 128
KNN = 3
BN_EPS = 1e-5
NT = N1 // 128          # 64 row tiles per core
NG = 16                 # groups of 4 tiles (512 rows)
GT = NT // NG           # tiles per group = 4
KAUG = 21               # augmented contraction rows (fp16 3-split pairs + sq2)
F32 = mybir.dt.float32
BF16 = mybir.dt.bfloat16
F16 = mybir.dt.float16
I8 = mybir.dt.int8
U32 = mybir.dt.uint32

# packed bf16 tensor layout (bf16-element offsets; f32/int8 sections bitcast)
SZ_F2 = N2 * C2 // 2            # feat2 int8 (first: indirect-DMA src offset 0)
SZ_F1 = 128 * N1 // 2           # feat1T as int8 bytes (2 int8 per bf16 slot)
SZ_XYZ1 = 3 * N1 * 2            # xyz1^T f32 (coord-major); splits built on device
SZ_XYZ2 = 3 * N2 * 2
SZ_W0 = 128 * 3 * MLP0          # W0^T fp16 chunks
SZ_W1 = 128 * 2 * MLP1          # W1^T fp16 chunks
SZ_SQ1 = 128 * NT * 2           # f32 as 2 bf16 slots
SZ_BNP0 = 128 * 4 * 2
SZ_BNP1 = 128 * 3 * 2           # gamma1, beta1, inv quant scale
OFF_F2 = 0
OFF_F1 = OFF_F2 + SZ_F2
OFF_XYZ1 = OFF_F1 + SZ_F1
OFF_XYZ2 = OFF_XYZ1 + SZ_XYZ1
OFF_W0 = OFF_XYZ2 + SZ_XYZ2
OFF_W1 = OFF_W0 + SZ_W0
OFF_SQ1 = OFF_W1 + SZ_W1
OFF_BNP0 = OFF_SQ1 + SZ_SQ1
OFF_BNP1 = OFF_BNP0 + SZ_BNP0
TOT16 = OFF_BNP1 + SZ_BNP1

_prog_cache = {}
_last_in_maps = None


def _host_prep(xyz1b, xyz2b, feat1b, feat2b, W0, W1, bnp0, bnp1):
    """Build the packed per-core input. Returns dict with one array.

    feat1/feat2 are quantized to int8 with per-channel scales; the scales are
    folded into W0's input columns, so the device sees raw int values (exact
    in bf16) and the matmul output is identical to using s*q floats.
    """
    import ml_dtypes
    bf = ml_dtypes.bfloat16
    s1 = np.maximum(np.abs(feat1b).max(0), 1e-12).astype(np.float32) / 127.0   # [128]
    q1 = np.clip(np.rint(feat1b / s1), -127, 127).astype(np.int8)              # [8192,128]
    s2 = np.maximum(np.abs(feat2b).max(0), 1e-12).astype(np.float32) / 127.0   # [256]
    q2 = np.clip(np.rint(feat2b / s2), -127, 127).astype(np.int8)              # [2048,256]
    W0s = W0 * np.concatenate([s1, s2])[None, :]                               # [256,384]
    w0t = np.ascontiguousarray(
        W0s.T.astype(np.float16).reshape(3, 128, MLP0).transpose(1, 0, 2))     # [128,3,256]
    w1t = np.ascontiguousarray(
        W1.T.astype(np.float16).reshape(2, 128, MLP1).transpose(1, 0, 2))      # [128,2,128]

    sq1 = (xyz1b.astype(np.float32) ** 2).sum(-1).astype(np.float32)
    sq1t = np.ascontiguousarray(sq1.reshape(NT, 128).T)  # [128, NT]
    feat1T8 = np.ascontiguousarray(q1.T)          # [128, N1] int8
    xyz1c = np.ascontiguousarray(xyz1b.T.astype(np.float32))  # [3, N1]
    xyz2c = np.ascontiguousarray(xyz2b.T.astype(np.float32))  # [3, N2]

    pk16 = np.empty((TOT16,), bf)
    pk16[OFF_F2:OFF_F2 + SZ_F2] = np.ascontiguousarray(q2).ravel().view(bf)
    pk16[OFF_F1:OFF_F1 + SZ_F1] = feat1T8.ravel().view(bf)
    pk16[OFF_XYZ1:OFF_XYZ1 + SZ_XYZ1] = xyz1c.ravel().view(bf)
    pk16[OFF_XYZ2:OFF_XYZ2 + SZ_XYZ2] = xyz2c.ravel().view(bf)
    pk16[OFF_W0:OFF_W0 + SZ_W0] = w0t.ravel().view(bf)
    pk16[OFF_W1:OFF_W1 + SZ_W1] = w1t.ravel().view(bf)
    pk16[OFF_SQ1:OFF_SQ1 + SZ_SQ1] = sq1t.ravel().view(bf)
    pk16[OFF_BNP0:OFF_BNP0 + SZ_BNP0] = bnp0.ravel().view(bf)
    pk16[OFF_BNP1:OFF_BNP1 + SZ_BNP1] = bnp1.ravel().view(bf)
    return {"pk16": pk16}


def _build_program(n_cores):
    nc = bacc.Bacc("TRN2", target_bir_lowering=False, debug=False)

    pk16_d = nc.dram_tensor("pk16", [TOT16], BF16, kind="ExternalInput")
    out_d = nc.dram_tensor("out", [N1, MLP1], mybir.dt.uint8, kind="ExternalOutput")

    feat2_v = pk16_d[OFF_F2:OFF_F2 + SZ_F2].bitcast(I8).rearrange("(a b) -> a b", b=C2)  # [2048, 256] int8
    feat1_v = pk16_d[OFF_F1:OFF_F1 + SZ_F1].bitcast(I8).rearrange("(a b) -> a b", b=N1)  # [128, 8192] int8
    xyz1c_v = pk16_d[OFF_XYZ1:OFF_XYZ1 + SZ_XYZ1].bitcast(F32).rearrange("(a b) -> a b", b=N1)  # [3, 8192]
    xyz2c_v = pk16_d[OFF_XYZ2:OFF_XYZ2 + SZ_XYZ2].bitcast(F32).rearrange("(a b) -> a b", b=N2)  # [3, 2048]
    w0t_v = pk16_d[OFF_W0:OFF_W0 + SZ_W0].bitcast(F16).rearrange("(a b c) -> a b c", b=3, c=MLP0)
    w1t_v = pk16_d[OFF_W1:OFF_W1 + SZ_W1].bitcast(F16).rearrange("(a b c) -> a b c", b=2, c=MLP1)
    sq1t_v = pk16_d[OFF_SQ1:OFF_SQ1 + SZ_SQ1].bitcast(F32).rearrange("(a b) -> a b", b=NT)
    bnp0_v = pk16_d[OFF_BNP0:OFF_BNP0 + SZ_BNP0].bitcast(F32).rearrange("(a b) -> a b", b=4)
    bnp1_v = pk16_d[OFF_BNP1:OFF_BNP1 + SZ_BNP1].bitcast(F32).rearrange("(a b) -> a b", b=3)

    NTOT = float(B * N1)  # total rows across cores for BN stats

    with tile.TileContext(nc) as tc:
        with (
            tc.tile_pool(name="const", bufs=1) as cpool,
            tc.tile_pool(name="karr", bufs=1) as kpool,
            tc.tile_pool(name="vps", bufs=1, space="PSUM") as vps_pool,
            tc.tile_pool(name="tps", bufs=2, space="PSUM") as tps_pool,
            tc.tile_pool(name="mps", bufs=1, space="PSUM") as mps_pool,
            tc.tile_pool(name="gbuf", bufs=2) as gpool,
            tc.tile_pool(name="xbuf", bufs=1) as xpool,
            tc.tile_pool(name="sbuf", bufs=2) as spool,
            tc.tile_pool(name="dram", bufs=1, space="DRAM") as dram,
        ):
            # ---- constants / persistent ----
            x1s = cpool.tile([KAUG, N1], F16)
            x2s = cpool.tile([KAUG, N2], F16)
            sq1t = cpool.tile([128, NT], F32)
            w0t = cpool.tile([128, 3, MLP0], F16)
            w1t = cpool.tile([128, 2, MLP1], F16)
            bnp0 = cpool.tile([128, 4], F32)
            bnp1 = cpool.tile([128, 3], F32)
            ident = cpool.tile([128, 128], F32)
            nc.sync.dma_start(sq1t[:], sq1t_v)
            nc.sync.dma_start(w0t[:], w0t_v)
            nc.sync.dma_start(w1t[:], w1t_v)
            nc.sync.dma_start(bnp0[:], bnp0_v)
            nc.sync.dma_start(bnp1[:], bnp1_v)
            from concourse.masks import make_identity
            make_identity(nc, ident[:])

            # ---- build the 21 augmented cdist rows from raw xyz on device ----
            # fp16 3-split per side: a+b+c ~= x (residual ~2^-33). Six pair
            # rows per coord recover 2<x1,x2> to fp32 ulp; pair layout per
            # coord (row = 6c+i):
            #   i: 0:(a1,d2) 1:(a1,e2) 2:(a1,f2) 3:(b1,d2) 4:(b1,e2) 5:(c1,d2)
            # rows 18-20: (ones, 3-split of -sq2)
            # Compute engines must start at partition 0, so each part is
            # computed in partition-0 staging tiles and DMA-placed into rows.
            CH = N1 // 2
            xyzrow = kpool.tile([1, CH], F32)
            resr = kpool.tile([1, CH], F32)
            ast = kpool.tile([1, CH], F16)
            bst = kpool.tile([1, CH], F16)
            cst = kpool.tile([1, CH], F16)
            sq2r = kpool.tile([1, N2], F32)
            onest = kpool.tile([1, CH], F16)
            nc.vector.memset(onest[:], 1.0)
            for c in range(3):
                r = 6 * c
                for h in range(2):
                    sl = slice(h * CH, (h + 1) * CH)
                    nc.sync.dma_start(xyzrow[:], xyz1c_v[c:c + 1, sl])
                    nc.vector.tensor_scalar(out=ast[:], in0=xyzrow[:], scalar1=2.0,
                                            scalar2=None, op0=mybir.AluOpType.mult)      # a1 = fp16(2x)
                    nc.vector.scalar_tensor_tensor(out=resr[:], in0=xyzrow[:], scalar=2.0, in1=ast[:],
                                                   op0=mybir.AluOpType.mult, op1=mybir.AluOpType.subtract)
                    nc.vector.tensor_copy(bst[:], resr[:])                               # b1
                    nc.vector.tensor_tensor(out=resr[:], in0=resr[:], in1=bst[:], op=mybir.AluOpType.subtract)
                    nc.vector.tensor_copy(cst[:], resr[:])                               # c1
                    for dst in (r, r + 1, r + 2):
                        nc.sync.dma_start(x1s[dst:dst + 1, sl], ast[:])
                    for dst in (r + 3, r + 4):
                        nc.sync.dma_start(x1s[dst:dst + 1, sl], bst[:])
                    nc.sync.dma_start(x1s[r + 5:r + 6, sl], cst[:])
                    if c == 0:
                        for dst in (18, 19, 20):
                            nc.sync.dma_start(x1s[dst:dst + 1, sl], onest[:])
            for c in range(3):
                r = 6 * c
                x2row = xyzrow[:, 0:N2]
                res2 = resr[:, 0:N2]
                tmp2 = resr[:, N2:2 * N2]
                a2 = ast[:, 0:N2]
                b2 = bst[:, 0:N2]
                c2 = cst[:, 0:N2]
                nc.sync.dma_start(x2row, xyz2c_v[c:c + 1, :])
                nc.vector.tensor_copy(a2, x2row)                                         # d2 = fp16(x)
                nc.vector.tensor_tensor(out=res2, in0=x2row, in1=a2, op=mybir.AluOpType.subtract)
                nc.vector.tensor_copy(b2, res2)                                          # e2
                nc.vector.tensor_tensor(out=res2, in0=res2, in1=b2, op=mybir.AluOpType.subtract)
                nc.vector.tensor_copy(c2, res2)                                          # f2
                if c == 0:
                    nc.vector.tensor_tensor(out=sq2r[:], in0=x2row, in1=x2row, op=mybir.AluOpType.mult)
                else:
                    nc.vector.tensor_tensor(out=tmp2, in0=x2row, in1=x2row, op=mybir.AluOpType.mult)
                    nc.vector.tensor_tensor(out=sq2r[:], in0=sq2r[:], in1=tmp2, op=mybir.AluOpType.add)
                for dst in (r, r + 3, r + 5):
                    nc.sync.dma_start(x2s[dst:dst + 1, :], a2)
                for dst in (r + 1, r + 4):
                    nc.sync.dma_start(x2s[dst:dst + 1, :], b2)
                nc.sync.dma_start(x2s[r + 2:r + 3, :], c2)
            a2 = ast[:, 0:N2]
            b2 = bst[:, 0:N2]
            c2 = cst[:, 0:N2]
            nc.vector.tensor_scalar(out=a2, in0=sq2r[:], scalar1=-1.0,
                                    scalar2=None, op0=mybir.AluOpType.mult)              # sa = fp16(-sq2)
            nc.vector.tensor_scalar(out=sq2r[:], in0=sq2r[:], scalar1=-1.0,
                                    scalar2=None, op0=mybir.AluOpType.mult)
            nc.vector.tensor_tensor(out=sq2r[:], in0=sq2r[:], in1=a2, op=mybir.AluOpType.subtract)
            nc.vector.tensor_copy(b2, sq2r[:])                                           # sb
            nc.vector.tensor_tensor(out=sq2r[:], in0=sq2r[:], in1=b2, op=mybir.AluOpType.subtract)
            nc.vector.tensor_copy(c2, sq2r[:])                                           # sc
            nc.sync.dma_start(x2s[18:19, :], a2)
            nc.sync.dma_start(x2s[19:20, :], b2)
            nc.sync.dma_start(x2s[20:21, :], c2)

            mv_all = cpool.tile([128, NT, 8], F32)
            mi_all = cpool.tile([128, NT, 8], U32)

            # ================= Phase 1: KNN =================
            for t in range(NT):
                v_ps = vps_pool.tile([128, N2], F32, tag="v")
                for j in range(4):
                    nc.tensor.matmul(
                        v_ps[:, j * 512:(j + 1) * 512],
                        x1s[:, t * 128:(t + 1) * 128],
                        x2s[:, j * 512:(j + 1) * 512],
                        start=True, stop=True,
                    )
                nc.vector.max(out=mv_all[:, t, :], in_=v_ps[:])
                nc.vector.max_index(out=mi_all[:, t, :], in_max=mv_all[:, t, :], in_values=v_ps[:])

            # ---- batched weight computation ----
            # d2 = sq1 - v   (v = 2cross - sq2)
            mv3 = mv_all[:, :, 0:KNN]                      # [128, NT, 3]
            d2 = kpool.tile([128, NT, KNN], F32)
            nc.vector.tensor_tensor(out=d2[:], in0=sq1t[:].to_broadcast([128, NT, KNN]),
                                    in1=mv3, op=mybir.AluOpType.subtract)
            nc.vector.tensor_scalar_max(d2[:], d2[:], 1e-12)
            dist = kpool.tile([128, NT, KNN], F32)
            nc.scalar.activation(out=dist[:], in_=d2[:], func=mybir.ActivationFunctionType.Sqrt)
            nc.vector.tensor_scalar_add(dist[:], dist[:], 1e-8)
            rr = kpool.tile([128, NT, KNN], F32)
            nc.vector.reciprocal(out=rr[:], in_=dist[:])
            rs = kpool.tile([128, NT, 1], F32)
            nc.vector.tensor_reduce(out=rs[:], in_=rr[:], axis=mybir.AxisListType.X, op=mybir.AluOpType.add)
            rsr = kpool.tile([128, NT, 1], F32)
            nc.vector.reciprocal(out=rsr[:], in_=rs[:])
            w_all = kpool.tile([128, NT, KNN], F32)
            nc.vector.tensor_tensor(out=w_all[:], in0=rr[:], in1=rsr[:].to_broadcast([128, NT, KNN]),
                                    op=mybir.AluOpType.mult)

            # contiguous per-k index arrays for indirect DMA offsets
            mi_k = kpool.tile([128, KNN, NT], U32)
            for k in range(KNN):
                nc.vector.tensor_copy(mi_k[:, k, :], mi_all[:, :, k])

            # ================= Phase 2: gather + interp + transposed MLP =================
            x0T = []
            for c in range(2):
                x0Tc = xpool.tile([128, N1], F16, tag=f"x0T{c}", name=f"x0T{c}")
                x0T.append(x0Tc)
            x1T = xpool.tile([128, N1], F16, tag="x1T")
            s1p0 = kpool.tile([128, 2, NG], F32)   # per-(chunk, group) sums of x0
            s2p0 = kpool.tile([128, 2, NG], F32)
            s1p1 = kpool.tile([128, NG], F32)
            s2p1 = kpool.tile([128, NG], F32)
            nc.vector.memset(s1p0[:], 0.0)
            nc.vector.memset(s2p0[:], 0.0)
            nc.vector.memset(s1p1[:], 0.0)
            nc.vector.memset(s2p1[:], 0.0)

            for g in range(NG):
                # gathers for this group's 4 tiles (one indirect DMA per (tile, k))
                gk = []
                for k in range(KNN):
                    gt = gpool.tile([128, GT, C2], I8, tag=f"g{k}", name=f"g{k}")
                    for j in range(GT):
                        t = g * GT + j
                        nc.gpsimd.indirect_dma_start(
                            out=gt[:, j, :],
                            out_offset=None,
                            in_=feat2_v,
                            in_offset=bass.IndirectOffsetOnAxis(ap=mi_k[:, k, t:t + 1], axis=0),
                        )
                    gk.append(gt)
                # feat1 int8 -> fp16 expansion (values <=127: exact);
                # per-channel dequant scales are folded into W0's columns.
                inT = gpool.tile([128, 3, 512], F16, tag="inT")
                f1i8 = gpool.tile([128, 512], I8, tag="f1i8")
                nc.sync.dma_start(f1i8[:], feat1_v[:, g * 512:(g + 1) * 512])
                nc.scalar.activation(out=inT[:, 0, :], in_=f1i8[:],
                                     func=mybir.ActivationFunctionType.Copy)
                # weighted interp per tile, then transpose to channel-major
                for j in range(GT):
                    t = g * GT + j
                    itp = gpool.tile([128, C2], F32, tag="itp")
                    nc.vector.tensor_scalar(out=itp[:], in0=gk[0][:, j, :], scalar1=w_all[:, t, 0:1],
                                            scalar2=None, op0=mybir.AluOpType.mult)
                    nc.vector.scalar_tensor_tensor(out=itp[:], in0=gk[1][:, j, :], scalar=w_all[:, t, 1:2],
                                                   in1=itp[:], op0=mybir.AluOpType.mult, op1=mybir.AluOpType.add)
                    nc.vector.scalar_tensor_tensor(out=itp[:], in0=gk[2][:, j, :], scalar=w_all[:, t, 2:3],
                                                   in1=itp[:], op0=mybir.AluOpType.mult, op1=mybir.AluOpType.add)
                    for c in range(2):
                        tp = tps_pool.tile([128, 128], F32, tag="tp")
                        nc.tensor.transpose(out=tp[:], in_=itp[:, c * 128:(c + 1) * 128], identity=ident[:])
                        nc.scalar.activation(out=inT[:, 1 + c, j * 128:(j + 1) * 128], in_=tp[:],
                                             func=mybir.ActivationFunctionType.Copy)

                # layer 0 matmuls: x0T chunk [128 out_ch, 512 rows]
                for c in range(2):
                    x0ps = mps_pool.tile([128, 512], F32, tag="x0ps")
                    for ki in range(3):
                        nc.tensor.matmul(
                            x0ps[:],
                            w0t[:, ki, c * 128:(c + 1) * 128],
                            inT[:, ki, :],
                            start=(ki == 0), stop=(ki == 2),
                        )
                    # S2 partial via ACT Square with accumulate; S1 fused into the copy
                    junk = spool.tile([128, 512], BF16, tag="junk")
                    nc.scalar.activation(out=junk[:], in_=x0ps[:], func=mybir.ActivationFunctionType.Square,
                                         accum_out=s2p0[:, c, g:g + 1])
                    nc.scalar.activation(out=x0T[c][:, g * 512:(g + 1) * 512], in_=x0ps[:],
                                         func=mybir.ActivationFunctionType.Copy,
                                         accum_out=s1p0[:, c, g:g + 1])

            # ---- BN0: reduce partials, AllReduce, compute affine ----
            st0 = kpool.tile([128, 4], F32)
            nc.vector.tensor_reduce(out=st0[:, 0:1], in_=s1p0[:, 0, :], axis=mybir.AxisListType.X, op=mybir.AluOpType.add)
            nc.vector.tensor_reduce(out=st0[:, 1:2], in_=s2p0[:, 0, :], axis=mybir.AxisListType.X, op=mybir.AluOpType.add)
            nc.vector.tensor_reduce(out=st0[:, 2:3], in_=s1p0[:, 1, :], axis=mybir.AxisListType.X, op=mybir.AluOpType.add)
            nc.vector.tensor_reduce(out=st0[:, 3:4], in_=s2p0[:, 1, :], axis=mybir.AxisListType.X, op=mybir.AluOpType.add)
            st0_in = dram.tile([128, 4], F32)
            st0_out = dram.tile([128, 4], F32)
            nc.gpsimd.dma_start(st0_in[:], st0[:])
            nc.gpsimd.collective_compute(
                "AllReduce", mybir.AluOpType.add,
                replica_groups=[list(range(n_cores))],
                ins=[st0_in.opt()], outs=[st0_out.opt()],
            )
            st0g = kpool.tile([128, 4], F32)
            nc.sync.dma_start(st0g[:], st0_out[:])
            # mean/var -> a = g*rsqrt(var+eps), bb = be - mean*a   (per chunk)
            ab0 = kpool.tile([128, 4], F32)   # a_c0, b_c0, a_c1, b_c1
            mean0 = kpool.tile([128, 2], F32)
            var0 = kpool.tile([128, 2], F32)
            sd0 = kpool.tile([128, 2], F32)
            m20 = kpool.tile([128, 2], F32)
            for c in range(2):
                nc.vector.tensor_scalar_mul(mean0[:, c:c + 1], st0g[:, 2 * c:2 * c + 1], 1.0 / NTOT)
                nc.vector.tensor_scalar_mul(var0[:, c:c + 1], st0g[:, 2 * c + 1:2 * c + 2], 1.0 / NTOT)
            nc.vector.tensor_tensor(out=m20[:], in0=mean0[:], in1=mean0[:], op=mybir.AluOpType.mult)
            nc.vector.tensor_tensor(out=var0[:], in0=var0[:], in1=m20[:], op=mybir.AluOpType.subtract)
            nc.vector.tensor_scalar_add(var0[:], var0[:], BN_EPS)
            nc.scalar.activation(out=sd0[:], in_=var0[:], func=mybir.ActivationFunctionType.Sqrt)
            nc.vector.reciprocal(out=sd0[:], in_=sd0[:])
            for c in range(2):
                nc.vector.tensor_tensor(out=ab0[:, 2 * c:2 * c + 1], in0=bnp0[:, 2 * c:2 * c + 1],
                                        in1=sd0[:, c:c + 1], op=mybir.AluOpType.mult)
                nc.vector.scalar_tensor_tensor(out=ab0[:, 2 * c + 1:2 * c + 2], in0=mean0[:, c:c + 1],
                                               scalar=-1.0, in1=ab0[:, 2 * c:2 * c + 1],
                                               op0=mybir.AluOpType.mult, op1=mybir.AluOpType.mult)
                nc.vector.tensor_tensor(out=ab0[:, 2 * c + 1:2 * c + 2], in0=ab0[:, 2 * c + 1:2 * c + 2],
                                        in1=bnp0[:, 2 * c + 1:2 * c + 2], op=mybir.AluOpType.add)

            # ---- layer 1 (+ BN1 stats) ----
            for g in range(NG):
                x0n = []
                for c in range(2):
                    x0nc = spool.tile([128, 512], F16, tag=f"x0n{c}", name=f"x0n{c}")
                    nc.scalar.activation(out=x0nc[:], in_=x0T[c][:, g * 512:(g + 1) * 512],
                                         func=mybir.ActivationFunctionType.Relu,
                                         scale=ab0[:, 2 * c:2 * c + 1], bias=ab0[:, 2 * c + 1:2 * c + 2])
                    x0n.append(x0nc)
                x1ps = mps_pool.tile([128, 512], F32, tag="x1ps")
                for c in range(2):
                    nc.tensor.matmul(x1ps[:], w1t[:, c, :], x0n[c][:], start=(c == 0), stop=(c == 1))
                junk = spool.tile([128, 512], BF16, tag="junk")
                nc.scalar.activation(out=junk[:], in_=x1ps[:], func=mybir.ActivationFunctionType.Square,
                                     accum_out=s2p1[:, g:g + 1])
                nc.scalar.activation(out=x1T[:, g * 512:(g + 1) * 512], in_=x1ps[:],
                                     func=mybir.ActivationFunctionType.Copy,
                                     accum_out=s1p1[:, g:g + 1])

            # ---- BN1 ----
            st1 = kpool.tile([128, 2], F32)
            nc.vector.tensor_reduce(out=st1[:, 0:1], in_=s1p1[:], axis=mybir.AxisListType.X, op=mybir.AluOpType.add)
            nc.vector.tensor_reduce(out=st1[:, 1:2], in_=s2p1[:], axis=mybir.AxisListType.X, op=mybir.AluOpType.add)
            st1_in = dram.tile([128, 2], F32)
            st1_out = dram.tile([128, 2], F32)
            nc.gpsimd.dma_start(st1_in[:], st1[:])
            nc.gpsimd.collective_compute(
                "AllReduce", mybir.AluOpType.add,
                replica_groups=[list(range(n_cores))],
                ins=[st1_in.opt()], outs=[st1_out.opt()],
            )
            st1g = kpool.tile([128, 2], F32)
            nc.sync.dma_start(st1g[:], st1_out[:])
            ab1 = kpool.tile([128, 2], F32)
            mean1 = kpool.tile([128, 1], F32)
            var1 = kpool.tile([128, 1], F32)
            nc.vector.tensor_scalar_mul(mean1[:], st1g[:, 0:1], 1.0 / NTOT)
            nc.vector.tensor_scalar_mul(var1[:], st1g[:, 1:2], 1.0 / NTOT)
            m21 = kpool.tile([128, 1], F32)
            nc.vector.tensor_tensor(out=m21[:], in0=mean1[:], in1=mean1[:], op=mybir.AluOpType.mult)
            nc.vector.tensor_tensor(out=var1[:], in0=var1[:], in1=m21[:], op=mybir.AluOpType.subtract)
            nc.vector.tensor_scalar_add(var1[:], var1[:], BN_EPS)
            nc.scalar.activation(out=var1[:], in_=var1[:], func=mybir.ActivationFunctionType.Sqrt)
            nc.vector.reciprocal(out=var1[:], in_=var1[:])
            nc.vector.tensor_tensor(out=ab1[:, 0:1], in0=bnp1[:, 0:1], in1=var1[:], op=mybir.AluOpType.mult)
            nc.vector.scalar_tensor_tensor(out=ab1[:, 1:2], in0=mean1[:], scalar=-1.0, in1=ab1[:, 0:1],
                                           op0=mybir.AluOpType.mult, op1=mybir.AluOpType.mult)
            nc.vector.tensor_tensor(out=ab1[:, 1:2], in0=ab1[:, 1:2], in1=bnp1[:, 1:2], op=mybir.AluOpType.add)
            # fold the uint8 quantization into the affine: relu(a*x+b)/s
            # == relu((a/s)x + b/s) for s > 0.
            nc.vector.tensor_tensor(out=ab1[:, 0:1], in0=ab1[:, 0:1], in1=bnp1[:, 2:3], op=mybir.AluOpType.mult)
            nc.vector.tensor_tensor(out=ab1[:, 1:2], in0=ab1[:, 1:2], in1=bnp1[:, 2:3], op=mybir.AluOpType.mult)

            # ---- BN1 apply + quantize + final transpose + output (uint8) ----
            for g in range(NG):
                x2t = spool.tile([128, 512], F32, tag="x2t")
                nc.scalar.activation(out=x2t[:], in_=x1T[:, g * 512:(g + 1) * 512],
                                     func=mybir.ActivationFunctionType.Relu,
                                     scale=ab1[:, 0:1], bias=ab1[:, 1:2])
                # round-to-nearest-integer via the fp32 magic number (+2^23
                # forces rounding at integer granularity), so the uint8
                # convert sees integral values and its rounding mode is moot;
                # clamp keeps it within [0, 255].
                nc.vector.tensor_scalar_min(x2t[:], x2t[:], 255.49)
                nc.vector.tensor_scalar_add(x2t[:], x2t[:], 8388608.0)
                nc.vector.tensor_scalar_add(x2t[:], x2t[:], -8388608.0)
                for j in range(GT):
                    t = g * GT + j
                    tp = tps_pool.tile([128, 128], F32, tag="tp")
                    nc.tensor.transpose(out=tp[:], in_=x2t[:, j * 128:(j + 1) * 128], identity=ident[:])
                    onat = spool.tile([128, 128], mybir.dt.uint8, tag="onat")
                    nc.vector.tensor_copy(onat[:], tp[:])
                    nc.sync.dma_start(out_d[t * 128:(t + 1) * 128, :], onat[:])

    nc.compile()
    return nc


def _get_program(n_cores):
    if n_cores not in _prog_cache:
        _prog_cache[n_cores] = _build_program(n_cores)
    return _prog_cache[n_cores]


def _prep_shared(gamma0, beta0, gamma1, beta1):
    bnp0 = np.stack([np.asarray(gamma0[:128]), np.asarray(beta0[:128]),
                     np.asarray(gamma0[128:]), np.asarray(beta0[128:])], 1).astype(np.float32)
    g1 = np.asarray(gamma1, np.float32)
    b1 = np.asarray(beta1, np.float32)
    # per-channel uint8 output scale: BN output is standardized, so
    # relu(g*xhat+b) <= 5.5|g| + max(b,0) (P(|xhat|>5.5) ~ 1e-8; rare
    # overshoots clamp at 255 with negligible L2 impact).
    s_col = (5.5 * np.abs(g1) + np.maximum(b1, 0.0) + 1e-12).astype(np.float32) / 255.0
    bnp1 = np.stack([g1, b1, 1.0 / s_col], 1).astype(np.float32)
    return bnp0, bnp1, s_col


def kernel(xyz1, xyz2, feat1, feat2, W0, b0, gamma0, beta0, W1, b1, gamma1, beta1):
    # note: b0/b1 cancel exactly inside train-mode BatchNorm -> ignored.
    xyz1 = np.asarray(xyz1, np.float32)
    xyz2 = np.asarray(xyz2, np.float32)
    feat1 = np.asarray(feat1, np.float32)
    feat2 = np.asarray(feat2, np.float32)
    W0 = np.asarray(W0, np.float32)
    W1 = np.asarray(W1, np.float32)
    bnp0, bnp1, s_col = _prep_shared(gamma0, beta0, gamma1, beta1)

    n_cores = B
    nc = _get_program(n_cores)
    in_maps = []
    for b in range(B):
        in_maps.append(_host_prep(xyz1[b], xyz2[b], feat1[b], feat2[b], W0, W1, bnp0, bnp1))

    global _last_in_maps
    _last_in_maps = in_maps
    res = run_bass_kernel_spmd(nc, in_maps, list(range(n_cores)))
    out = np.stack([res.results[b]["out"] for b in range(B)], 0).astype(np.float32)
    out *= s_col[None, None, :]
    return out
